# revision 10
# baseline (speedup 1.0000x reference)
"""BandhaAttention Trainium2 kernel.

Sharding: 8 cores = 2 (batch) x 4 (head groups of 4 heads).
Per core: qkv projection for its 4 heads (q/k produced transposed, v natural),
gated q, causal attention via transposed scores (tk on partitions), exp on ACT,
AV with V-stationary matmuls (ones column -> softmax sums for free),
normalization via gpsimd partition_broadcast, out-projection row-sharded.
Host sums the 4 partial outputs per batch.
"""

import os
import sys

import numpy as np

for p in ("/opt/trn_rl_repo", "/opt/trn_rl_repo/concourse"):
    if p not in sys.path and os.path.isdir(p):
        sys.path.insert(0, p)

import ml_dtypes

import concourse.bacc as bacc
import concourse.mybir as mybir
from concourse.bass_utils import run_bass_kernel_spmd
from concourse.tile import TileContext

BF16 = mybir.dt.bfloat16
F32 = mybir.dt.float32
AF = mybir.ActivationFunctionType

T = 2048
D = 1024
HD = 64
NH_LOC = 4      # heads per core
DL = NH_LOC * HD  # 256 local qkv channels
KT = D // 128   # 8 contraction chunks
NQ = T // 512   # 4 tq chunks of 512
NTT = T // 128  # 16 tiles of 128

TALA = [5, 6, 7, 8]

LAST = None  # last BassKernelResults (for profiling from test.py)


def build_nc():
    nc = bacc.Bacc("TRN2", target_bir_lowering=False)
    xt_d = nc.dram_tensor("xt", [D, T], BF16, kind="ExternalInput")
    wqk_d = nc.dram_tensor("wqk", [D, 2 * DL], BF16, kind="ExternalInput")
    wv_d = nc.dram_tensor("wv", [D, DL], BF16, kind="ExternalInput")
    wout_d = nc.dram_tensor("wout", [DL, D], BF16, kind="ExternalInput")
    gate_d = nc.dram_tensor("gate", [DL, T], BF16, kind="ExternalInput")
    tri_d = nc.dram_tensor("tri", [128, 128], BF16, kind="ExternalInput")
    out_d = nc.dram_tensor("out", [T, D], F32, kind="ExternalOutput")

    with TileContext(nc) as tc:
        with (
            tc.tile_pool(name="pers", bufs=2) as pers,
            tc.tile_pool(name="pc1", bufs=1) as pc1,
            tc.tile_pool(name="pv", bufs=NTT) as pv,
        ):
            # ---- constants ----
            tri = pc1.tile([128, 128], BF16, tag="tri", name="tri")
            nc.sync.dma_start(tri, tri_d[:, :])
            wout_sb = []
            for c in range(2):
                w = pers.tile([128, D], BF16, tag="wout", name="wout_sb")
                nc.sync.dma_start(w, wout_d[c * 128:(c + 1) * 128, :])
                wout_sb.append(w)

            # persistent products of phase 1
            qp_sb = [pers.tile([128, T], BF16, tag="qp", name="qp_sb")
                     for _ in range(2)]
            kp_sb = [pers.tile([128, T], BF16, tag="kp", name="kp_sb")
                     for _ in range(2)]
            v_all = [pv.tile([128, NH_LOC * (HD + 1)], BF16, tag="vall",
                             name="v_all") for _ in range(NTT)]
            aoT = [pers.tile([128, T], BF16, tag="aoT", name="aoT")
                   for _ in range(2)]

            # ---- phase 1 + 2 interleaved ----
            with (
                tc.tile_pool(name="pin", bufs=KT) as pin,
                tc.tile_pool(name="pexp", bufs=2) as pexp,
                tc.tile_pool(name="poex", bufs=3) as poex,
                tc.tile_pool(name="psm", bufs=2) as psm,
                tc.tile_pool(name="pstg", bufs=2) as pstg,
                tc.tile_pool(name="psq", bufs=2, space="PSUM") as psq,
                tc.tile_pool(name="pst", bufs=2, space="PSUM") as pst,
                tc.tile_pool(name="pav", bufs=2, space="PSUM") as pav,
            ):
                gate_sb = []
                for c in range(2):
                    g = pin.tile([128, T], BF16, tag="gate", name="gate_sb",
                                 bufs=2)
                    nc.sync.dma_start(g, gate_d[c * 128:(c + 1) * 128, :])
                    gate_sb.append(g)
                xt_sb, wqk_sb, wv_sb = [], [], []
                for kc in range(KT):
                    xt = pin.tile([128, T], BF16, tag="xt", name="xt_sb")
                    nc.sync.dma_start(xt, xt_d[kc * 128:(kc + 1) * 128, :])
                    xt_sb.append(xt)
                    wqk = pin.tile([128, 2 * DL], BF16, tag="wqk", name="wqk_sb")
                    nc.sync.dma_start(wqk, wqk_d[kc * 128:(kc + 1) * 128, :])
                    wqk_sb.append(wqk)
                    wv = pin.tile([128, DL], BF16, tag="wv", name="wv_sb")
                    nc.sync.dma_start(wv, wv_d[kc * 128:(kc + 1) * 128, :])
                    wv_sb.append(wv)

                def do_qk(m, n0, n1):  # m-tile of qT/kT, tq chunks [n0,n1)
                    dst = qp_sb[m] if m < 2 else kp_sb[m - 2]
                    for n in range(n0, n1):
                        ps = psq.tile([128, 512], F32, tag="psq", name="ps_qk")
                        for kc in range(KT):
                            nc.tensor.matmul(
                                ps,
                                lhsT=wqk_sb[kc][:, m * 128:(m + 1) * 128],
                                rhs=xt_sb[kc][:, n * 512:(n + 1) * 512],
                                start=(kc == 0), stop=(kc == KT - 1),
                            )
                        if m < 2:  # gate the queries while evacuating
                            nc.vector.tensor_mul(
                                dst[:, n * 512:(n + 1) * 512], ps,
                                gate_sb[m][:, n * 512:(n + 1) * 512])
                        else:
                            nc.vector.tensor_copy(
                                dst[:, n * 512:(n + 1) * 512], ps)

                def do_v(t):  # v natural t-tile (128, 256) -> v_all
                    ps = psq.tile([128, DL], F32, tag="psq", name="ps_v")
                    for kc in range(KT):
                        nc.tensor.matmul(
                            ps,
                            lhsT=xt_sb[kc][:, t * 128:(t + 1) * 128],
                            rhs=wv_sb[kc],
                            start=(kc == 0), stop=(kc == KT - 1),
                        )
                    src = ps.rearrange("p (h c) -> p h c", c=HD)
                    dst = v_all[t].rearrange("p (h c) -> p h c", c=HD + 1)
                    nc.vector.tensor_copy(dst[:, :, 0:HD], src)
                    nc.vector.memset(dst[:, :, HD:HD + 1], 1.0)

                expt = {}  # (h, i) -> tile covering tq cols [128*i, T)

                def do_st(p, i):
                    w_i = T - 128 * i
                    e0 = pexp.tile([128, w_i], BF16, tag=f"e{i}", name="e0")
                    e1 = pexp.tile([128, w_i], BF16, tag=f"e{i}", name="e1")
                    expt[(2 * p, i)] = e0
                    expt[(2 * p + 1, i)] = e1
                    for c0 in range(0, w_i, 1024):  # psum pieces of <=1024
                        w = min(1024, w_i - c0)
                        sts = []
                        for hh in range(2):
                            st = pst.tile([128, 1024], F32, tag="st",
                                          name="st_ps")
                            lo, hi = hh * 64, hh * 64 + 64
                            for nn in range(0, w, 512):
                                wn = min(512, w - nn)
                                a = 128 * i + c0 + nn
                                nc.tensor.matmul(
                                    st[:, nn:nn + wn],
                                    lhsT=kp_sb[p][lo:hi, i * 128:(i + 1) * 128],
                                    rhs=qp_sb[p][lo:hi, a:a + wn],
                                    start=True, stop=True,
                                )
                            sts.append(st)
                        for hh, st in enumerate(sts):
                            e = expt[(2 * p + hh, i)]
                            nc.scalar.activation(
                                e[:, c0:c0 + w], st[:, 0:w], AF.Exp,
                                scale=0.125)
                        if c0 == 0:  # causal band mask on leading 128 cols
                            for hh in range(2):
                                e = expt[(2 * p + hh, i)]
                                nc.vector.tensor_mul(e[:, 0:128],
                                                     e[:, 0:128], tri)

                def do_av(p, hh, j):
                    h = 2 * p + hh
                    av = pav.tile([128, 512], F32, tag="av", name="av_ps")
                    last_i = 4 * j + 3
                    for i in range(last_i + 1):
                        off = 512 * j - 128 * i
                        r = max(0, -off)  # 128*(i%4) on diagonal tiles
                        nc.tensor.matmul(
                            av[0:HD + 1, r:512],
                            lhsT=v_all[i][:, hh * 65 + p * 130:
                                          hh * 65 + p * 130 + 65],
                            rhs=expt[(h, i)][:, off + r:off + 512],
                            start=(i == 0), stop=(i == last_i),
                        )
                    oex = poex.tile([HD + 1, 512], F32, tag="oex", name="oex")
                    nc.vector.tensor_copy(oex, av[0:HD + 1, :])
                    rc = psm.tile([1, 512], F32, tag="rc", name="rc_sb")
                    nc.vector.reciprocal(rc, oex[HD:HD + 1, :])
                    bc = psm.tile([64, 512], F32, tag="bc", name="bc_sb")
                    nc.gpsimd.partition_broadcast(bc, rc)
                    nc.vector.tensor_mul(
                        aoT[p][hh * 64:hh * 64 + 64, j * 512:(j + 1) * 512],
                        oex[0:HD, :], bc)

                def do_proj(t):
                    for n in range(2):
                        po = pav.tile([128, 512], F32, tag="av", name="po_ps")
                        for c in range(2):
                            nc.tensor.matmul(
                                po,
                                lhsT=aoT[c][:, t * 128:(t + 1) * 128],
                                rhs=wout_sb[c][:, n * 512:(n + 1) * 512],
                                start=(c == 0), stop=(c == 1),
                            )
                        stg = pstg.tile([128, 512], F32, tag="stg", name="stg_sb")
                        nc.vector.tensor_copy(stg, po)
                        nc.sync.dma_start(
                            out_d[t * 128:(t + 1) * 128,
                                  n * 512:(n + 1) * 512], stg)

                # pair-0 q/k projection first
                do_qk(0, 0, 4)
                do_qk(2, 0, 4)
                # pair-0 attention, with v and pair-1 q/k as PE filler
                fill = [(1, 0, 2), (1, 2, 4), (3, 0, 2), (3, 2, 4)]
                for j in range(NQ):
                    if j > 0:
                        do_av(0, 0, j - 1)
                        do_av(0, 1, j - 1)
                    for t in range(4 * j, 4 * j + 4):
                        do_v(t)
                    do_qk(*fill[j])
                    do_st(0, 4 * j)
                    do_st(0, 4 * j + 1)
                    do_st(0, 4 * j + 2)
                    do_st(0, 4 * j + 3)
                do_av(0, 0, 3)
                do_av(0, 1, 3)
                # pair-1 attention with projection as filler
                for j in range(NQ):
                    if j > 0:
                        do_av(1, 0, j - 1)
                        do_av(1, 1, j - 1)
                        for t in range(4 * (j - 1), 4 * j):
                            do_proj(t)
                    do_st(1, 4 * j)
                    do_st(1, 4 * j + 1)
                    do_st(1, 4 * j + 2)
                    do_st(1, 4 * j + 3)
                do_av(1, 0, 3)
                do_av(1, 1, 3)
                for t in range(12, 16):
                    do_proj(t)
    nc.compile()
    return nc


def _prep_inputs(x, w_qkv, w_out, bandha_gate):
    bf = ml_dtypes.bfloat16
    t = np.arange(T)
    gate_full = np.empty((16, T), np.float64)
    for h in range(16):
        cyc = TALA[h % len(TALA)]
        gate_full[h] = 1.0 / (1.0 + np.exp(-bandha_gate[h, t % cyc].astype(np.float64)))
    tri = (np.arange(128)[None, :] >= np.arange(128)[:, None]).astype(bf)

    in_maps = []
    for c in range(8):
        b, g = c // 4, c % 4
        xt = np.ascontiguousarray(x[b].T).astype(bf)
        wqk = np.concatenate(
            [w_qkv[:, g * DL:(g + 1) * DL],
             w_qkv[:, D + g * DL:D + (g + 1) * DL]], axis=1).astype(bf)
        wv = np.ascontiguousarray(w_qkv[:, 2 * D + g * DL:2 * D + (g + 1) * DL]).astype(bf)
        wout = np.ascontiguousarray(w_out[g * DL:(g + 1) * DL, :]).astype(bf)
        gb = np.repeat(gate_full[4 * g:4 * g + 4].astype(np.float32), HD, axis=0).astype(bf)
        in_maps.append({"xt": xt, "wqk": wqk, "wv": wv, "wout": wout,
                        "gate": np.ascontiguousarray(gb), "tri": tri})
    return in_maps


def kernel(**inputs):
    global LAST
    x = np.asarray(inputs["x"], np.float32)
    w_qkv = np.asarray(inputs["w_qkv"], np.float32)
    w_out = np.asarray(inputs["w_out"], np.float32)
    bandha_gate = np.asarray(inputs["bandha_gate"], np.float32)

    in_maps = _prep_inputs(x, w_qkv, w_out, bandha_gate)
    nc = build_nc()
    res = run_bass_kernel_spmd(
        nc, in_maps, core_ids=list(range(8)),
        trace=os.environ.get("BANDHA_TRACE") == "1",
    )
    LAST = res
    outs = [r["out"] for r in res.results]
    full = np.empty((2, T, D), np.float32)
    for b in range(2):
        full[b] = outs[4 * b] + outs[4 * b + 1] + outs[4 * b + 2] + outs[4 * b + 3]
    return full
